# revision 23
# baseline (speedup 1.0000x reference)
"""AmplifiedAttention Trainium2 kernel (8 NeuronCores, SPMD).

Key algebraic simplification: rotate_half(q)·rotate_half(k) == q·k and
(rotate_half(q)^2)·(rotate_half(k)^2) == q^2·k^2, so the reference's second
"rotated" attention pass is bit-for-bit (up to fp assoc.) identical to the
first: out = out1 + HADAMARD_SCALE * out1^2 * gate_w.

Sharding: 16 heads -> 2 heads/core. Each core projects Q/K/V for its heads
over the full (batch*seq) axis, runs causal attention with a fused
second-order score term ([rope(Q); sqrt(lam)*rope(Q)^2] contraction), then an
AllToAll redistributes attention outputs so each core computes a 512-row
slice of the final output projection against the full Wo.
"""

import math
import os

import numpy as np
import ml_dtypes

import concourse.bass as bass
import concourse.bacc as bacc
import concourse.mybir as mybir
from concourse.tile import TileContext
from concourse.bass_utils import run_bass_kernel_spmd

BF16 = mybir.dt.bfloat16
F32 = mybir.dt.float32

B, S, D = 2, 2048, 1024
H = 16
HD = D // H            # 64
NC = 8                 # cores
NHPC = H // NC         # 2 heads per core
SEQ = B * S            # 4096
NK = D // 128          # 8 contraction chunks
NNT = SEQ // 512       # 8 seq 512-tiles
NST = SEQ // 128       # 32 seq 128-tiles
NQT = S // 512         # 4 query 512-tiles per (b,h)
NTC = S // 128         # 16 key 128-chunks per (b,h)

LAMBDA = 0.1
HADAMARD_SCALE = 0.05
ROPE_BASE = 10000.0
INV_SQRT_HD = 1.0 / math.sqrt(HD)
# Q weights are pre-scaled by INV_SQRT_HD on the host.  Q-tilde bottom rows
# need sqrt(lam)*INV_SQRT_HD*ropeQ^2 computed from the pre-scaled ropeQ:
#   (lam^0.25 / sqrt(INV_SQRT_HD) * x)^2 = sqrt(lam)/INV_SQRT_HD * x^2
Q_SQ_SCALE = LAMBDA ** 0.25 / math.sqrt(INV_SQRT_HD)
K_SQ_SCALE = LAMBDA ** 0.25

_GRAPH = None


def _emit(nc, tc, t):
    """Emit the per-core program. t: dict name -> DRAM tensor handle."""
    AF = mybir.ActivationFunctionType
    OP = mybir.AluOpType
    singles = tc.alloc_tile_pool(name="singles", bufs=1)

    # ---- constant / input loads ----
    cos_sb = singles.tile([128, S], BF16, tag="cos", name="cos")
    sin_sb = singles.tile([128, S], BF16, tag="sin", name="sin")
    masks_sb = singles.tile([128, 512], BF16, tag="masks", name="masks")
    gate_sb = singles.tile([64, 1], F32, tag="gate", name="gate")
    wq_sb = singles.tile([128, NK, 256], BF16, tag="wq", name="wq")
    wk_sb = singles.tile([128, NK, 256], BF16, tag="wk", name="wk")
    wv_sb = singles.tile([128, NK, 128], BF16, tag="wv", name="wv")
    wo_sb = singles.tile([128, NK, 1024], BF16, tag="wo", name="wo")
    xt_sb = singles.tile([128, NK, SEQ], BF16, tag="xt", name="xt")

    nc.sync.dma_start(out=wq_sb[:], in_=t["wq2"].ap().rearrange("k p m -> p k m"))
    nc.sync.dma_start(out=wk_sb[:], in_=t["wk2"].ap().rearrange("k p m -> p k m"))
    nc.sync.dma_start(out=wv_sb[:], in_=t["wv"].ap().rearrange("k p m -> p k m"))
    nc.scalar.dma_start(out=cos_sb[:], in_=t["cost"].ap())
    nc.scalar.dma_start(out=sin_sb[:], in_=t["sint"].ap())
    nc.scalar.dma_start(out=masks_sb[:], in_=t["masks"].ap())
    nc.scalar.dma_start(out=gate_sb[:], in_=t["gate"].ap())
    # xt split by (chunk, seq-half): batch-0 halves first so batch-0
    # projections can start after 4MB instead of 8MB.
    for k in range(NK):
        nc.sync.dma_start(out=xt_sb[:, k, 0:S], in_=t["xt"].ap()[k, :, 0:S])
    for k in range(NK):
        nc.sync.dma_start(out=xt_sb[:, k, S:SEQ], in_=t["xt"].ap()[k, :, S:SEQ])
    nc.scalar.dma_start(out=wo_sb[:], in_=t["wo"].ap().rearrange("k p j -> p k j"))

    # ---- persistent stage-1 outputs ----
    # qt/kt per (b, h): [128, S]: rows 0:64 = rope (Q pre-scaled by 1/sqrt(hd)),
    # rows 64:128 = scaled square.
    qt = [[singles.tile([128, S], BF16, tag=f"qt{b}{h}", name=f"qt{b}{h}") for h in range(NHPC)] for b in range(B)]
    kt = [[singles.tile([128, S], BF16, tag=f"kt{b}{h}", name=f"kt{b}{h}") for h in range(NHPC)] for b in range(B)]
    # v per batch: [128, 16, 256]: per 128-seq-chunk: [ones(0:64), h0(64:128), ones(128:192), h1(192:256)]
    vt = [singles.tile([128, NTC, 256], BF16, tag=f"v{b}", name=f"v{b}") for b in range(B)]
    att = [singles.tile([128, S], BF16, tag=f"att{b}", name=f"att{b}") for b in range(B)]

    for b in range(B):
        ones_ap = bass.AP(
            tensor=vt[b].tensor, offset=vt[b].offset,
            ap=[vt[b].ap[0], [256, NTC], [128, 2], [1, 64]],
        )
        nc.vector.memset(ones_ap, 1.0)

    pools = {}

    def emit_proj_n(b, j2):
        """Q/K/Qs/Ks projections + rope + V for one 512-seq tile of batch b."""
        ps_proj = pools["ps_proj"]
        if True:
            n = NQT * b + j2
            ns = slice(512 * n, 512 * n + 512)
            cs = slice(512 * j2, 512 * j2 + 512)
            for (w2, dst) in ((wq_sb, qt), (wk_sb, kt)):
                ps_a = ps_proj.tile([128, 512], F32, tag="pp", name="pp")
                ps_s = ps_proj.tile([128, 512], F32, tag="pp", name="pp")
                for k in range(NK):
                    nc.tensor.matmul(ps_a[:], w2[:, k, 0:128], xt_sb[:, k, ns],
                                     start=(k == 0), stop=(k == NK - 1))
                for k in range(NK):
                    nc.tensor.matmul(ps_s[:], w2[:, k, 128:256], xt_sb[:, k, ns],
                                     start=(k == 0), stop=(k == NK - 1))
                pc = rope_tmp.tile([128, 512], BF16, tag="pc", name="pc")
                psn = rope_tmp.tile([128, 512], BF16, tag="psn", name="psn")
                nc.vector.tensor_tensor(out=pc[:], in0=ps_a[:], in1=cos_sb[:, cs], op=OP.mult)
                nc.vector.tensor_tensor(out=psn[:], in0=ps_s[:], in1=sin_sb[:, cs], op=OP.mult)
                nc.vector.tensor_tensor(out=dst[b][0][0:64, cs], in0=pc[0:64, :], in1=psn[0:64, :], op=OP.add)
                nc.vector.tensor_tensor(out=dst[b][1][0:64, cs], in0=pc[64:128, :], in1=psn[64:128, :], op=OP.add)
            # V for the 4 seq-128-chunks of this n-tile
            pv = ps_proj.tile([128, 512], F32, tag="pp", name="pp")
            for st4 in range(4):
                st = 4 * n + st4
                for k in range(NK):
                    nc.tensor.matmul(pv[:, 128 * st4:128 * st4 + 128],
                                     xt_sb[:, k, 128 * st:128 * st + 128], wv_sb[:, k, :],
                                     start=(k == 0), stop=(k == NK - 1))
            for st4 in range(4):
                st4g = 4 * j2 + st4
                vdst = bass.AP(
                    tensor=vt[b].tensor, offset=vt[b].offset + 256 * st4g + 64,
                    ap=[vt[b].ap[0], [128, 2], [1, 64]],
                )
                nc.scalar.copy(vdst, pv[:, 128 * st4:128 * st4 + 128].rearrange("p (a b) -> p a b", a=2))

    def emit_squares(b):
        for h in range(NHPC):
            nc.vector.scalar_tensor_tensor(
                out=qt[b][h][64:128, :], in0=qt[b][h][0:64, :], scalar=Q_SQ_SCALE ** 2,
                in1=qt[b][h][0:64, :], op0=OP.mult, op1=OP.mult)
            nc.vector.scalar_tensor_tensor(
                out=kt[b][h][64:128, :], in0=kt[b][h][0:64, :], scalar=K_SQ_SCALE ** 2,
                in1=kt[b][h][0:64, :], op0=OP.mult, op1=OP.mult)

    def emit_att_j(b, h, j):
        """Attention for one (batch, head, q-tile): scores -> exp/mask -> A@V -> gate."""
        ps_s, ps_av, a_pool, nrm = pools["ps_s"], pools["ps_av"], pools["a_pool"], pools["nrm"]
        QT, KT = qt[b][h], kt[b][h]
        if True:
            qs = slice(512 * j, 512 * j + 512)
            I = 4 * j + 4  # t-chunks (causal keep)
            po = ps_av.tile([128, 512], F32, tag="po", name="po")
            # pass 1: scores + exp + mask for all t-chunk pairs, so the
            # PE streams score matmuls without stalling on ACT's exp
            a_list = []
            for ip in range(I // 2):
                i0, i1 = 2 * ip, 2 * ip + 1
                pss = ps_s.tile([128, 1024], F32, tag="pss", name="pss")
                nc.tensor.matmul(pss[:, 0:512], KT[:, 128 * i0:128 * i0 + 128], QT[:, qs],
                                 start=True, stop=True)
                nc.tensor.matmul(pss[:, 512:1024], KT[:, 128 * i1:128 * i1 + 128], QT[:, qs],
                                 start=True, stop=True)
                a = a_pool.tile([128, 1024], BF16, tag="a", name="a")
                if i0 < 4 * j:
                    nc.scalar.activation(out=a[:], in_=pss[:], func=AF.Exp)
                else:
                    # diagonal pair: only cols >= 128*o are causally valid;
                    # exp the valid range, zero the rest, mask the boundary.
                    for half, i in ((0, i0), (1, i1)):
                        o = i - 4 * j
                        lo, hi = 512 * half, 512 * half + 512
                        if o > 0:
                            nc.vector.memset(a[:, lo:lo + 128 * o], 0.0)
                        nc.scalar.activation(out=a[:, lo + 128 * o:hi],
                                             in_=pss[:, lo + 128 * o:hi], func=AF.Exp)
                        nc.vector.tensor_tensor(
                            out=a[:, lo + 128 * o:hi], in0=a[:, lo + 128 * o:hi],
                            in1=masks_sb[:, 0:512 - 128 * o], op=OP.mult)
                a_list.append(a)
            # pass 2: A@V accumulation
            for ip, a in enumerate(a_list):
                i0, i1 = 2 * ip, 2 * ip + 1
                nc.tensor.matmul(po[:], vt[b][:, i0, 128 * h:128 * h + 128], a[:, 0:512],
                                 start=(ip == 0), stop=False)
                nc.tensor.matmul(po[:], vt[b][:, i1, 128 * h:128 * h + 128], a[:, 512:1024],
                                 start=False, stop=(ip == I // 2 - 1))
            # rows 0:64 of po = softmax denominators (replicated), 64:128 = A@V
            rd = nrm.tile([64, 512], F32, tag="rd", name="rd")
            m = nrm.tile([64, 512], BF16, tag="m", name="m")
            sq = nrm.tile([64, 512], BF16, tag="sq", name="sq")
            nc.vector.reciprocal_approx_fast(out=rd[:], in_=po[0:64, :])
            nc.vector.tensor_tensor(out=m[:], in0=po[64:128, :], in1=rd[:], op=OP.mult)
            nc.vector.tensor_tensor(out=sq[:], in0=m[:], in1=m[:], op=OP.mult)
            nc.vector.scalar_tensor_tensor(
                out=att[b][64 * h:64 * h + 64, 512 * j:512 * j + 512],
                in0=sq[:], scalar=gate_sb[:, 0:1], in1=m[:],
                op0=OP.mult, op1=OP.add)

    def emit_ccin(b, j):
        c = 4 * b + j
        nc.sync.dma_start(
            out=t["cc_in"].ap()[128 * c:128 * c + 128, :],
            in_=att[b][:, 512 * j:512 * j + 512])

    def emit_a2a():
        nc.gpsimd.collective_compute(
            "AllToAll", OP.bypass,
            replica_groups=[list(range(NC))],
            ins=[t["cc_in"].ap()], outs=[t["cc_out"].ap()],
        )

    def emit_oproj():
        # each core computes its own 512 seq rows x full D, K = all 1024 head
        # dims gathered via the A2A (chunk k = global dims 128k).
        ga = singles.tile([128, NK, 512], BF16, tag="ga", name="ga")
        nc.sync.dma_start(out=ga[:], in_=t["cc_out"].ap().rearrange("(k p) q -> p k q", p=128))
        for m4 in range(4):
            osb = pools["ob"].tile([128, 1024], F32, tag="osb", name="osb")
            for jj in range(2):
                poo = pools["ps_o"].tile([128, 512], F32, tag="poo", name="poo")
                for k in range(NK):
                    nc.tensor.matmul(poo[:], ga[:, k, 128 * m4:128 * m4 + 128],
                                     wo_sb[:, k, 512 * jj:512 * jj + 512],
                                     start=(k == 0), stop=(k == NK - 1))
                nc.vector.tensor_copy(osb[:, 512 * jj:512 * jj + 512], poo[:])
            nc.sync.dma_start(out=t["out"].ap()[128 * m4:128 * m4 + 128, :], in_=osb[:])

    # ---- schedule ----
    # SBUF pools live for the whole kernel (LIFO stack bottom).
    pools["rope_tmp"] = rope_tmp = tc.alloc_tile_pool(name="rope_tmp", bufs=2)
    pools["a_pool"] = a_pool = tc.alloc_tile_pool(name="a_pool", bufs=8)
    pools["nrm"] = nrm = tc.alloc_tile_pool(name="nrm", bufs=2)
    pools["ob"] = ob = tc.alloc_tile_pool(name="ob", bufs=2)

    # scope A: projections b0 (6 PSUM banks, deep pipelining)
    pools["ps_proj"] = ps_proj_a = tc.alloc_tile_pool(name="ps_proj", bufs=6, space="PSUM")
    for j2 in range(NQT):
        emit_proj_n(0, j2)
    emit_squares(0)
    ps_proj_a.release()

    # scope B: attention b0 interleaved with projections b1 (ACT-heavy exp
    # overlaps PE-heavy projections); 4 + 2 + 2 = 8 banks
    pools["ps_s"] = ps_s = tc.alloc_tile_pool(name="ps_s", bufs=2, space="PSUM")
    pools["ps_av"] = ps_av = tc.alloc_tile_pool(name="ps_av", bufs=2, space="PSUM")
    pools["ps_proj"] = ps_proj_b = tc.alloc_tile_pool(name="ps_proj2", bufs=2, space="PSUM")
    for j2 in range(NQT):
        emit_att_j(0, 0, j2)
        emit_proj_n(1, j2)
    emit_squares(1)
    for j in range(NQT):
        emit_att_j(0, 1, j)
        emit_ccin(0, j)
    ps_proj_b.release()

    # scope C: attention b1, single A2A, output projection
    pools["ps_o"] = ps_o = tc.alloc_tile_pool(name="ps_o", bufs=2, space="PSUM")
    for j in range(NQT):
        emit_att_j(1, 0, j)
    for j in range(NQT):
        emit_att_j(1, 1, j)
        emit_ccin(1, j)
    emit_a2a()
    emit_oproj()

    ps_o.release()
    ps_av.release()
    ps_s.release()
    ob.release()
    nrm.release()
    a_pool.release()
    rope_tmp.release()
    singles.release()


def build_graph():
    nc = bacc.Bacc("TRN2", target_bir_lowering=False, debug=False, num_devices=NC)
    t = {}
    t["xt"] = nc.dram_tensor("xt", [NK, 128, SEQ], BF16, kind="ExternalInput")
    t["wq2"] = nc.dram_tensor("wq2", [NK, 128, 256], BF16, kind="ExternalInput")
    t["wk2"] = nc.dram_tensor("wk2", [NK, 128, 256], BF16, kind="ExternalInput")
    t["wv"] = nc.dram_tensor("wv", [NK, 128, 128], BF16, kind="ExternalInput")
    t["wo"] = nc.dram_tensor("wo", [NK, 128, 1024], BF16, kind="ExternalInput")
    t["cost"] = nc.dram_tensor("cost", [128, S], BF16, kind="ExternalInput")
    t["sint"] = nc.dram_tensor("sint", [128, S], BF16, kind="ExternalInput")
    t["masks"] = nc.dram_tensor("masks", [128, 512], BF16, kind="ExternalInput")
    t["gate"] = nc.dram_tensor("gate", [64, 1], F32, kind="ExternalInput")
    t["out"] = nc.dram_tensor("out", [SEQ // NC, D], F32, kind="ExternalOutput")
    t["cc_in"] = nc.dram_tensor("cc_in", [NC * 128, 512], BF16)
    t["cc_out"] = nc.dram_tensor("cc_out", [NC * 128, 512], BF16)
    with TileContext(nc) as tc:
        _emit(nc, tc, t)
    nc.compile()
    return nc


def _bf16(a):
    return np.asarray(a, dtype=np.float32).astype(ml_dtypes.bfloat16)


def _shift_sign(w):
    """Rows p: p%64<32 -> -w[p+32]; else +w[p-32] (within each 64-row head block)."""
    out = np.empty_like(w)
    for h0 in range(0, w.shape[0], 64):
        out[h0:h0 + 32] = -w[h0 + 32:h0 + 64]
        out[h0 + 32:h0 + 64] = w[h0:h0 + 32]
    return out


def host_prep(x, Wq, Wk, Wv, Wo, gate_w):
    x = np.asarray(x, np.float32)
    Wq = np.asarray(Wq, np.float32)
    Wk = np.asarray(Wk, np.float32)
    Wv = np.asarray(Wv, np.float32)
    Wo = np.asarray(Wo, np.float32)
    gate_w = np.asarray(gate_w, np.float32)

    xt = _bf16(np.ascontiguousarray(x.reshape(SEQ, D).T).reshape(NK, 128, SEQ))
    wo = _bf16(np.ascontiguousarray(Wo.T).reshape(NK, 128, D))

    half = HD // 2
    inv_freq = 1.0 / (ROPE_BASE ** (np.arange(half, dtype=np.float32) / half))
    ang = np.arange(S, dtype=np.float32)[:, None] * inv_freq[None, :]  # [S, 32]
    cos_f = np.cos(ang)  # [S, 32]
    sin_f = np.sin(ang)
    p32 = np.arange(128) % 32
    nmod = np.arange(S)
    cost = _bf16(cos_f[nmod[None, :], p32[:, None]])
    sint = _bf16(sin_f[nmod[None, :], p32[:, None]])

    p = np.arange(128)[:, None]
    qp = np.arange(512)[None, :]
    masks = _bf16((p <= qp).astype(np.float32))

    gate = (HADAMARD_SCALE * gate_w).astype(np.float32).reshape(64, 1)

    in_maps = []
    for c in range(NC):
        hs = slice(128 * c, 128 * c + 128)
        wq_s = Wq[hs] * INV_SQRT_HD
        wk_s = Wk[hs]
        wq2 = np.concatenate([
            np.ascontiguousarray(wq_s.T).reshape(NK, 128, 128),
            np.ascontiguousarray(_shift_sign(wq_s).T).reshape(NK, 128, 128),
        ], axis=2)
        wk2 = np.concatenate([
            np.ascontiguousarray(wk_s.T).reshape(NK, 128, 128),
            np.ascontiguousarray(_shift_sign(wk_s).T).reshape(NK, 128, 128),
        ], axis=2)
        wv_c = np.ascontiguousarray(Wv[hs].T).reshape(NK, 128, 128)
        in_maps.append({
            "xt": xt, "wq2": _bf16(wq2), "wk2": _bf16(wk2), "wv": _bf16(wv_c),
            "wo": wo, "cost": cost, "sint": sint, "masks": masks, "gate": gate,
        })
    return in_maps


def _install_ntff_shim():
    """The agent image's antenv lacks axon_hooks; recreate it so
    run_bass_kernel_spmd(trace=True) can capture an NTFF profile."""
    import sys
    import types
    if "antenv.axon_hooks" in sys.modules:
        return True
    try:
        import antenv  # noqa: F401
        from trn_agent_boot.trn_boot import _ntff_profile_via_ctypes
        mod = types.ModuleType("antenv.axon_hooks")
        mod._hook = None
        mod.set_axon_ntff_profile_hook = lambda h: setattr(mod, "_hook", h)
        mod.get_axon_ntff_profile_hook = lambda: mod._hook
        sys.modules["antenv.axon_hooks"] = mod
        mod.set_axon_ntff_profile_hook(_ntff_profile_via_ctypes("/opt/axon/libaxon_pjrt.so"))
        import concourse.bass_utils as bu
        bu.upload_artifacts = lambda tmpdir: str(tmpdir)
        return True
    except Exception:
        return False


def kernel(x, Wq, Wk, Wv, Wo, gate_w):
    global _GRAPH
    if _GRAPH is None:
        _GRAPH = build_graph()
    in_maps = host_prep(x, Wq, Wk, Wv, Wo, gate_w)
    trace = bool(os.environ.get("KERNEL_TRACE")) and _install_ntff_shim()
    res = run_bass_kernel_spmd(_GRAPH, in_maps, core_ids=list(range(NC)), trace=trace)
    if trace and res.exec_time_ns is not None:
        print(f"HW exec time: {res.exec_time_ns} ns")
        kernel.last_exec_time_ns = res.exec_time_ns
        kernel.last_profile = res
    out = np.concatenate([res.results[c]["out"] for c in range(NC)], axis=0)
    return out.reshape(B, S, D)


# revision 24
# speedup vs baseline: 1.1262x; 1.1262x over previous
"""AmplifiedAttention Trainium2 kernel (8 NeuronCores, SPMD).

Key algebraic simplification: rotate_half(q)·rotate_half(k) == q·k and
(rotate_half(q)^2)·(rotate_half(k)^2) == q^2·k^2, so the reference's second
"rotated" attention pass is bit-for-bit (up to fp assoc.) identical to the
first: out = out1 + HADAMARD_SCALE * out1^2 * gate_w.

Sharding: 16 heads -> 2 heads/core. Each core projects Q/K/V for its heads
over the full (batch*seq) axis, runs causal attention with a fused
second-order score term ([rope(Q); sqrt(lam)*rope(Q)^2] contraction), then an
AllToAll redistributes attention outputs so each core computes a 512-row
slice of the final output projection against the full Wo.
"""

import math
import os

import numpy as np
import ml_dtypes

import concourse.bass as bass
import concourse.bacc as bacc
import concourse.mybir as mybir
from concourse.tile import TileContext
from concourse.bass_utils import run_bass_kernel_spmd

BF16 = mybir.dt.bfloat16
F32 = mybir.dt.float32

B, S, D = 2, 2048, 1024
H = 16
HD = D // H            # 64
NC = 8                 # cores
NHPC = H // NC         # 2 heads per core
SEQ = B * S            # 4096
NK = D // 128          # 8 contraction chunks
NNT = SEQ // 512       # 8 seq 512-tiles
NST = SEQ // 128       # 32 seq 128-tiles
NQT = S // 512         # 4 query 512-tiles per (b,h)
NTC = S // 128         # 16 key 128-chunks per (b,h)

LAMBDA = 0.1
HADAMARD_SCALE = 0.05
ROPE_BASE = 10000.0
INV_SQRT_HD = 1.0 / math.sqrt(HD)
# Q weights are pre-scaled by INV_SQRT_HD on the host.  Q-tilde bottom rows
# need sqrt(lam)*INV_SQRT_HD*ropeQ^2 computed from the pre-scaled ropeQ:
#   (lam^0.25 / sqrt(INV_SQRT_HD) * x)^2 = sqrt(lam)/INV_SQRT_HD * x^2
Q_SQ_SCALE = LAMBDA ** 0.25 / math.sqrt(INV_SQRT_HD)
K_SQ_SCALE = LAMBDA ** 0.25

_GRAPH = None


def _emit(nc, tc, t):
    """Emit the per-core program. t: dict name -> DRAM tensor handle."""
    AF = mybir.ActivationFunctionType
    OP = mybir.AluOpType
    singles = tc.alloc_tile_pool(name="singles", bufs=1)

    # ---- constant / input loads ----
    cos_sb = singles.tile([128, S], BF16, tag="cos", name="cos")
    sin_sb = singles.tile([128, S], BF16, tag="sin", name="sin")
    masks_sb = singles.tile([128, 512], BF16, tag="masks", name="masks")
    gate_sb = singles.tile([64, 1], F32, tag="gate", name="gate")
    wq_sb = singles.tile([128, NK, 256], BF16, tag="wq", name="wq")
    wk_sb = singles.tile([128, NK, 256], BF16, tag="wk", name="wk")
    wv_sb = singles.tile([128, NK, 128], BF16, tag="wv", name="wv")
    wo_sb = singles.tile([128, NK, 1024], BF16, tag="wo", name="wo")
    xt_sb = singles.tile([128, NK, SEQ], BF16, tag="xt", name="xt")

    nc.sync.dma_start(out=wq_sb[:], in_=t["wq2"].ap().rearrange("k p m -> p k m"))
    nc.sync.dma_start(out=wk_sb[:], in_=t["wk2"].ap().rearrange("k p m -> p k m"))
    nc.sync.dma_start(out=wv_sb[:], in_=t["wv"].ap().rearrange("k p m -> p k m"))
    nc.scalar.dma_start(out=cos_sb[:], in_=t["cost"].ap())
    nc.scalar.dma_start(out=sin_sb[:], in_=t["sint"].ap())
    nc.scalar.dma_start(out=masks_sb[:], in_=t["masks"].ap())
    nc.scalar.dma_start(out=gate_sb[:], in_=t["gate"].ap())
    # xt split by (chunk, seq-half): batch-0 halves first so batch-0
    # projections can start after 4MB instead of 8MB.
    for k in range(NK):
        nc.sync.dma_start(out=xt_sb[:, k, 0:S], in_=t["xt"].ap()[k, :, 0:S])
    for k in range(NK):
        nc.sync.dma_start(out=xt_sb[:, k, S:SEQ], in_=t["xt"].ap()[k, :, S:SEQ])
    nc.scalar.dma_start(out=wo_sb[:], in_=t["wo"].ap().rearrange("k p j -> p k j"))

    # ---- persistent stage-1 outputs ----
    # qt/kt per (b, h): [128, S]: rows 0:64 = rope (Q pre-scaled by 1/sqrt(hd)),
    # rows 64:128 = scaled square.
    qt = [[singles.tile([128, S], BF16, tag=f"qt{b}{h}", name=f"qt{b}{h}") for h in range(NHPC)] for b in range(B)]
    kt = [[singles.tile([128, S], BF16, tag=f"kt{b}{h}", name=f"kt{b}{h}") for h in range(NHPC)] for b in range(B)]
    # v per batch: [128, 16, 256]: per 128-seq-chunk: [ones(0:64), h0(64:128), ones(128:192), h1(192:256)]
    vt = [singles.tile([128, NTC, 256], BF16, tag=f"v{b}", name=f"v{b}") for b in range(B)]
    att = [singles.tile([128, S], BF16, tag=f"att{b}", name=f"att{b}") for b in range(B)]

    for b in range(B):
        ones_ap = bass.AP(
            tensor=vt[b].tensor, offset=vt[b].offset,
            ap=[vt[b].ap[0], [256, NTC], [128, 2], [1, 64]],
        )
        nc.vector.memset(ones_ap, 1.0)

    pools = {}

    def emit_proj_n(b, j2):
        """Q/K/Qs/Ks projections + rope + V for one 512-seq tile of batch b."""
        ps_proj = pools["ps_proj"]
        if True:
            n = NQT * b + j2
            ns = slice(512 * n, 512 * n + 512)
            cs = slice(512 * j2, 512 * j2 + 512)
            for (w2, dst) in ((wq_sb, qt), (wk_sb, kt)):
                ps_a = ps_proj.tile([128, 512], F32, tag="pp", name="pp")
                ps_s = ps_proj.tile([128, 512], F32, tag="pp", name="pp")
                for k in range(NK):
                    nc.tensor.matmul(ps_a[:], w2[:, k, 0:128], xt_sb[:, k, ns],
                                     start=(k == 0), stop=(k == NK - 1))
                for k in range(NK):
                    nc.tensor.matmul(ps_s[:], w2[:, k, 128:256], xt_sb[:, k, ns],
                                     start=(k == 0), stop=(k == NK - 1))
                pc = rope_tmp.tile([128, 512], BF16, tag="pc", name="pc")
                psn = rope_tmp.tile([128, 512], BF16, tag="psn", name="psn")
                nc.vector.tensor_tensor(out=pc[:], in0=ps_a[:], in1=cos_sb[:, cs], op=OP.mult)
                nc.vector.tensor_tensor(out=psn[:], in0=ps_s[:], in1=sin_sb[:, cs], op=OP.mult)
                nc.vector.tensor_tensor(out=dst[b][0][0:64, cs], in0=pc[0:64, :], in1=psn[0:64, :], op=OP.add)
                nc.vector.tensor_tensor(out=dst[b][1][0:64, cs], in0=pc[64:128, :], in1=psn[64:128, :], op=OP.add)
            # V for the 4 seq-128-chunks of this n-tile
            pv = ps_proj.tile([128, 512], F32, tag="pp", name="pp")
            for st4 in range(4):
                st = 4 * n + st4
                for k in range(NK):
                    nc.tensor.matmul(pv[:, 128 * st4:128 * st4 + 128],
                                     xt_sb[:, k, 128 * st:128 * st + 128], wv_sb[:, k, :],
                                     start=(k == 0), stop=(k == NK - 1))
            for st4 in range(4):
                st4g = 4 * j2 + st4
                vdst = bass.AP(
                    tensor=vt[b].tensor, offset=vt[b].offset + 256 * st4g + 64,
                    ap=[vt[b].ap[0], [128, 2], [1, 64]],
                )
                nc.scalar.copy(vdst, pv[:, 128 * st4:128 * st4 + 128].rearrange("p (a b) -> p a b", a=2))

    def emit_squares(b):
        for h in range(NHPC):
            nc.vector.scalar_tensor_tensor(
                out=qt[b][h][64:128, :], in0=qt[b][h][0:64, :], scalar=Q_SQ_SCALE ** 2,
                in1=qt[b][h][0:64, :], op0=OP.mult, op1=OP.mult)
            nc.vector.scalar_tensor_tensor(
                out=kt[b][h][64:128, :], in0=kt[b][h][0:64, :], scalar=K_SQ_SCALE ** 2,
                in1=kt[b][h][0:64, :], op0=OP.mult, op1=OP.mult)

    def emit_att_j(b, h, j):
        """Attention for one (batch, head, q-tile): scores -> exp/mask -> A@V -> gate."""
        ps_s, ps_av, a_pool, nrm = pools["ps_s"], pools["ps_av"], pools["a_pool"], pools["nrm"]
        QT, KT = qt[b][h], kt[b][h]
        if True:
            qs = slice(512 * j, 512 * j + 512)
            I = 4 * j + 4  # t-chunks (causal keep)
            po = ps_av.tile([128, 512], F32, tag="po", name="po")
            # pass 1: scores + exp + mask for all t-chunk pairs, so the
            # PE streams score matmuls without stalling on ACT's exp
            a_list = []
            for ip in range(I // 2):
                i0, i1 = 2 * ip, 2 * ip + 1
                pss = ps_s.tile([128, 1024], F32, tag="pss", name="pss")
                nc.tensor.matmul(pss[:, 0:512], KT[:, 128 * i0:128 * i0 + 128], QT[:, qs],
                                 start=True, stop=True)
                nc.tensor.matmul(pss[:, 512:1024], KT[:, 128 * i1:128 * i1 + 128], QT[:, qs],
                                 start=True, stop=True)
                a = a_pool.tile([128, 1024], BF16, tag="a", name="a")
                if i0 < 4 * j:
                    nc.scalar.activation(out=a[:], in_=pss[:], func=AF.Exp)
                else:
                    # diagonal pair: only cols >= 128*o are causally valid;
                    # exp the valid range, zero the rest, mask the boundary.
                    for half, i in ((0, i0), (1, i1)):
                        o = i - 4 * j
                        lo, hi = 512 * half, 512 * half + 512
                        if o > 0:
                            nc.vector.memset(a[:, lo:lo + 128 * o], 0.0)
                        nc.scalar.activation(out=a[:, lo + 128 * o:hi],
                                             in_=pss[:, lo + 128 * o:hi], func=AF.Exp)
                        nc.vector.tensor_tensor(
                            out=a[:, lo + 128 * o:hi], in0=a[:, lo + 128 * o:hi],
                            in1=masks_sb[:, 0:512 - 128 * o], op=OP.mult)
                a_list.append(a)
            # pass 2: A@V accumulation
            for ip, a in enumerate(a_list):
                i0, i1 = 2 * ip, 2 * ip + 1
                nc.tensor.matmul(po[:], vt[b][:, i0, 128 * h:128 * h + 128], a[:, 0:512],
                                 start=(ip == 0), stop=False)
                nc.tensor.matmul(po[:], vt[b][:, i1, 128 * h:128 * h + 128], a[:, 512:1024],
                                 start=False, stop=(ip == I // 2 - 1))
            # rows 0:64 of po = softmax denominators (replicated), 64:128 = A@V
            rd = nrm.tile([64, 512], F32, tag="rd", name="rd")
            m = nrm.tile([64, 512], BF16, tag="m", name="m")
            sq = nrm.tile([64, 512], BF16, tag="sq", name="sq")
            nc.vector.reciprocal_approx_fast(out=rd[:], in_=po[0:64, :])
            nc.vector.tensor_tensor(out=m[:], in0=po[64:128, :], in1=rd[:], op=OP.mult)
            nc.vector.tensor_tensor(out=sq[:], in0=m[:], in1=m[:], op=OP.mult)
            nc.vector.scalar_tensor_tensor(
                out=att[b][64 * h:64 * h + 64, 512 * j:512 * j + 512],
                in0=sq[:], scalar=gate_sb[:, 0:1], in1=m[:],
                op0=OP.mult, op1=OP.add)

    def emit_ccin(b, j):
        c = 4 * b + j
        nc.sync.dma_start(
            out=t["cc_in"].ap()[128 * c:128 * c + 128, :],
            in_=att[b][:, 512 * j:512 * j + 512])

    def emit_a2a():
        nc.gpsimd.collective_compute(
            "AllToAll", OP.bypass,
            replica_groups=[list(range(NC))],
            ins=[t["cc_in"].ap()], outs=[t["cc_out"].ap()],
        )

    def emit_oproj():
        # each core computes its own 512 seq rows x full D, K = all 1024 head
        # dims gathered via the A2A (chunk k = global dims 128k).
        ga = singles.tile([128, NK, 512], BF16, tag="ga", name="ga")
        nc.sync.dma_start(out=ga[:], in_=t["cc_out"].ap().rearrange("(k p) q -> p k q", p=128))
        for m4 in range(4):
            osb = pools["ob"].tile([128, 1024], F32, tag="osb", name="osb")
            for jj in range(2):
                poo = pools["ps_av"].tile([128, 512], F32, tag="po", name="poo")
                for k in range(NK):
                    nc.tensor.matmul(poo[:], ga[:, k, 128 * m4:128 * m4 + 128],
                                     wo_sb[:, k, 512 * jj:512 * jj + 512],
                                     start=(k == 0), stop=(k == NK - 1))
                nc.vector.tensor_copy(osb[:, 512 * jj:512 * jj + 512], poo[:])
            nc.sync.dma_start(out=t["out"].ap()[128 * m4:128 * m4 + 128, :], in_=osb[:])

    # ---- schedule ----
    # SBUF pools live for the whole kernel (LIFO stack bottom).
    pools["rope_tmp"] = rope_tmp = tc.alloc_tile_pool(name="rope_tmp", bufs=2)
    pools["a_pool"] = a_pool = tc.alloc_tile_pool(name="a_pool", bufs=8)
    pools["nrm"] = nrm = tc.alloc_tile_pool(name="nrm", bufs=2)
    pools["ob"] = ob = tc.alloc_tile_pool(name="ob", bufs=2)

    # scope A: all projections (6 PSUM banks, deep pipelining)
    pools["ps_proj"] = ps_proj_a = tc.alloc_tile_pool(name="ps_proj", bufs=6, space="PSUM")
    for j2 in range(NQT):
        emit_proj_n(0, j2)
    emit_squares(0)
    for j2 in range(NQT):
        emit_proj_n(1, j2)
    emit_squares(1)
    ps_proj_a.release()

    # scope B: attention (ps_s 6 banks + ps_av 2 = 8); oproj reuses ps_av
    pools["ps_s"] = ps_s = tc.alloc_tile_pool(name="ps_s", bufs=3, space="PSUM")
    pools["ps_av"] = ps_av = tc.alloc_tile_pool(name="ps_av", bufs=2, space="PSUM")
    for j in range(NQT):
        emit_att_j(0, 0, j)
    for j in range(NQT):
        emit_att_j(0, 1, j)
        emit_ccin(0, j)
    for j in range(NQT):
        emit_att_j(1, 0, j)
    for j in range(NQT):
        emit_att_j(1, 1, j)
        emit_ccin(1, j)
    emit_a2a()
    emit_oproj()

    ps_av.release()
    ps_s.release()
    ob.release()
    nrm.release()
    a_pool.release()
    rope_tmp.release()
    singles.release()


def build_graph():
    nc = bacc.Bacc("TRN2", target_bir_lowering=False, debug=False, num_devices=NC)
    t = {}
    t["xt"] = nc.dram_tensor("xt", [NK, 128, SEQ], BF16, kind="ExternalInput")
    t["wq2"] = nc.dram_tensor("wq2", [NK, 128, 256], BF16, kind="ExternalInput")
    t["wk2"] = nc.dram_tensor("wk2", [NK, 128, 256], BF16, kind="ExternalInput")
    t["wv"] = nc.dram_tensor("wv", [NK, 128, 128], BF16, kind="ExternalInput")
    t["wo"] = nc.dram_tensor("wo", [NK, 128, 1024], BF16, kind="ExternalInput")
    t["cost"] = nc.dram_tensor("cost", [128, S], BF16, kind="ExternalInput")
    t["sint"] = nc.dram_tensor("sint", [128, S], BF16, kind="ExternalInput")
    t["masks"] = nc.dram_tensor("masks", [128, 512], BF16, kind="ExternalInput")
    t["gate"] = nc.dram_tensor("gate", [64, 1], F32, kind="ExternalInput")
    t["out"] = nc.dram_tensor("out", [SEQ // NC, D], F32, kind="ExternalOutput")
    t["cc_in"] = nc.dram_tensor("cc_in", [NC * 128, 512], BF16)
    t["cc_out"] = nc.dram_tensor("cc_out", [NC * 128, 512], BF16)
    with TileContext(nc) as tc:
        _emit(nc, tc, t)
    nc.compile()
    return nc


def _bf16(a):
    return np.asarray(a, dtype=np.float32).astype(ml_dtypes.bfloat16)


def _shift_sign(w):
    """Rows p: p%64<32 -> -w[p+32]; else +w[p-32] (within each 64-row head block)."""
    out = np.empty_like(w)
    for h0 in range(0, w.shape[0], 64):
        out[h0:h0 + 32] = -w[h0 + 32:h0 + 64]
        out[h0 + 32:h0 + 64] = w[h0:h0 + 32]
    return out


def host_prep(x, Wq, Wk, Wv, Wo, gate_w):
    x = np.asarray(x, np.float32)
    Wq = np.asarray(Wq, np.float32)
    Wk = np.asarray(Wk, np.float32)
    Wv = np.asarray(Wv, np.float32)
    Wo = np.asarray(Wo, np.float32)
    gate_w = np.asarray(gate_w, np.float32)

    xt = _bf16(np.ascontiguousarray(x.reshape(SEQ, D).T).reshape(NK, 128, SEQ))
    wo = _bf16(np.ascontiguousarray(Wo.T).reshape(NK, 128, D))

    half = HD // 2
    inv_freq = 1.0 / (ROPE_BASE ** (np.arange(half, dtype=np.float32) / half))
    ang = np.arange(S, dtype=np.float32)[:, None] * inv_freq[None, :]  # [S, 32]
    cos_f = np.cos(ang)  # [S, 32]
    sin_f = np.sin(ang)
    p32 = np.arange(128) % 32
    nmod = np.arange(S)
    cost = _bf16(cos_f[nmod[None, :], p32[:, None]])
    sint = _bf16(sin_f[nmod[None, :], p32[:, None]])

    p = np.arange(128)[:, None]
    qp = np.arange(512)[None, :]
    masks = _bf16((p <= qp).astype(np.float32))

    gate = (HADAMARD_SCALE * gate_w).astype(np.float32).reshape(64, 1)

    in_maps = []
    for c in range(NC):
        hs = slice(128 * c, 128 * c + 128)
        wq_s = Wq[hs] * INV_SQRT_HD
        wk_s = Wk[hs]
        wq2 = np.concatenate([
            np.ascontiguousarray(wq_s.T).reshape(NK, 128, 128),
            np.ascontiguousarray(_shift_sign(wq_s).T).reshape(NK, 128, 128),
        ], axis=2)
        wk2 = np.concatenate([
            np.ascontiguousarray(wk_s.T).reshape(NK, 128, 128),
            np.ascontiguousarray(_shift_sign(wk_s).T).reshape(NK, 128, 128),
        ], axis=2)
        wv_c = np.ascontiguousarray(Wv[hs].T).reshape(NK, 128, 128)
        in_maps.append({
            "xt": xt, "wq2": _bf16(wq2), "wk2": _bf16(wk2), "wv": _bf16(wv_c),
            "wo": wo, "cost": cost, "sint": sint, "masks": masks, "gate": gate,
        })
    return in_maps


def _install_ntff_shim():
    """The agent image's antenv lacks axon_hooks; recreate it so
    run_bass_kernel_spmd(trace=True) can capture an NTFF profile."""
    import sys
    import types
    if "antenv.axon_hooks" in sys.modules:
        return True
    try:
        import antenv  # noqa: F401
        from trn_agent_boot.trn_boot import _ntff_profile_via_ctypes
        mod = types.ModuleType("antenv.axon_hooks")
        mod._hook = None
        mod.set_axon_ntff_profile_hook = lambda h: setattr(mod, "_hook", h)
        mod.get_axon_ntff_profile_hook = lambda: mod._hook
        sys.modules["antenv.axon_hooks"] = mod
        mod.set_axon_ntff_profile_hook(_ntff_profile_via_ctypes("/opt/axon/libaxon_pjrt.so"))
        import concourse.bass_utils as bu
        bu.upload_artifacts = lambda tmpdir: str(tmpdir)
        return True
    except Exception:
        return False


def kernel(x, Wq, Wk, Wv, Wo, gate_w):
    global _GRAPH
    if _GRAPH is None:
        _GRAPH = build_graph()
    in_maps = host_prep(x, Wq, Wk, Wv, Wo, gate_w)
    trace = bool(os.environ.get("KERNEL_TRACE")) and _install_ntff_shim()
    res = run_bass_kernel_spmd(_GRAPH, in_maps, core_ids=list(range(NC)), trace=trace)
    if trace and res.exec_time_ns is not None:
        print(f"HW exec time: {res.exec_time_ns} ns")
        kernel.last_exec_time_ns = res.exec_time_ns
        kernel.last_profile = res
    out = np.concatenate([res.results[c]["out"] for c in range(NC)], axis=0)
    return out.reshape(B, S, D)
